# revision 5
# baseline (speedup 1.0000x reference)
"""Trainium2 Bass kernel for the CIntegration embedding-lookup module.

reference semantics (all fp32):
    ct    = concat(one_hot(rgap, 32), one_hot(sgap, 32), one_hot(pcount, 64))  # [B,S,128]
    Cct   = W.T[rgap] + W.T[32+sgap] + W.T[64+pcount]                          # [B,S,128]
    theta = vt * Cct
    out   = concat(theta, ct)                                                  # [B,S,256]

Strategy (8 NeuronCores, data-parallel over the batch dim, W replicated):
  - per core: 32 batch rows = 32768 tokens, processed in 32 chunks of 1024
    tokens. SBUF partition p holds tokens {8p+j, j=0..7} of the chunk so all
    DMAs are fully contiguous per partition.
  - transposed one-hot ctT[bin, tok] built on-chip: a K=3 matmul broadcasts
    the (offset) indices across partitions, then one is_equal against a
    partition-iota yields ctT in bf16 (exact, values 0/1).
  - the 3-row gather of W.T is ctT.T @ W.T, computed as matmuls with the
    one-hot as the stationary operand; W.T is fed as bf16 hi+lo halves
    accumulated in PSUM (exact to ~2^-17 relative).
  - ct (token-major one-hot for the output) is ctT run through the PE
    transpose, then a ScalarE copy (bf16->f32) into the staging tile.
  - theta = vt * Cct is one VectorE multiply per 512 tokens.
  - staging is a [128, 2048] f32 tile per chunk -> a single 1 MiB store DMA.
"""

import numpy as np

B, S, EMB = 256, 1024, 128
NUM_RGAP, NUM_SGAP, NUM_PCOUNT = 32, 32, 64
NTOTAL = NUM_RGAP + NUM_SGAP + NUM_PCOUNT  # 128
NCORES = 8
ROWS_PER_CORE = B // NCORES                # 32
T_CORE = ROWS_PER_CORE * S                 # 32768 tokens per core
CHUNK = 1024                               # tokens per chunk
NCHUNK = T_CORE // CHUNK                   # 32
JT = CHUNK // 128                          # 8 token-tiles per chunk
HALF = 512                                 # tokens per PSUM round
NH = CHUNK // HALF                         # 2 halves per chunk

_compiled = {}


def _build_program(repeats=1):
    import concourse.bacc as bacc
    import concourse.mybir as mybir
    from concourse import tile

    f32 = mybir.dt.float32
    bf16 = mybir.dt.bfloat16
    Alu = mybir.AluOpType

    nc = bacc.Bacc(None)

    vt_in = nc.declare_dram_parameter("vt", [NCHUNK, 128, CHUNK], f32, isOutput=False)
    idx3_in = nc.declare_dram_parameter("idx3", [3, T_CORE], bf16, isOutput=False)
    whi_in = nc.declare_dram_parameter("w_hi", [128, 128], bf16, isOutput=False)
    wlo_in = nc.declare_dram_parameter("w_lo", [128, 128], bf16, isOutput=False)
    sel_in = nc.declare_dram_parameter("sel3", [3, 128], bf16, isOutput=False)
    iota_in = nc.declare_dram_parameter("iota_col", [128, 1], f32, isOutput=False)
    id_in = nc.declare_dram_parameter("ident", [128, 128], bf16, isOutput=False)
    out_ext = nc.declare_dram_parameter(
        "out", [NCHUNK, 128, 2 * CHUNK], f32, isOutput=True
    )

    with tile.TileContext(nc) as tc:
        with (
            tc.tile_pool(name="consts", bufs=1) as consts,
            tc.tile_pool(name="vt", bufs=3) as vtp,
            tc.tile_pool(name="stage", bufs=3) as stp,
            tc.tile_pool(name="ctT", bufs=2) as ctp,
            tc.tile_pool(name="ps_b3", bufs=2, space="PSUM") as psb3,
            tc.tile_pool(name="ps_ct", bufs=2, space="PSUM") as psct,
            tc.tile_pool(name="ps_cc", bufs=2, space="PSUM") as pscc,
        ):
            whi = consts.tile([128, 128], bf16, tag="whi")
            wlo = consts.tile([128, 128], bf16, tag="wlo")
            sel3 = consts.tile([3, 128], bf16, tag="sel3")
            iota = consts.tile([128, 1], f32, tag="iota")
            ident = consts.tile([128, 128], bf16, tag="ident")
            idx3 = consts.tile([3, T_CORE], bf16, tag="idx3")
            nc.sync.dma_start(out=whi[:, :], in_=whi_in[:, :])
            nc.sync.dma_start(out=wlo[:, :], in_=wlo_in[:, :])
            nc.sync.dma_start(out=sel3[:, :], in_=sel_in[:, :])
            nc.sync.dma_start(out=iota[:, :], in_=iota_in[:, :])
            nc.sync.dma_start(out=ident[:, :], in_=id_in[:, :])
            nc.sync.dma_start(out=idx3[:, :], in_=idx3_in[:, :])

            for c in [ci for _ in range(repeats) for ci in range(NCHUNK)]:
                vt_sb = vtp.tile([128, CHUNK], f32, tag="vt")
                nc.sync.dma_start(out=vt_sb[:, :], in_=vt_in[c])
                stage = stp.tile([128, 2 * CHUNK], f32, tag="stage")
                st3 = stage[:, :].rearrange("p (j f) -> p j f", f=256)
                ctT = ctp.tile([128, CHUNK], bf16, tag="ctT")

                for h in range(NH):
                    t0 = c * CHUNK + h * HALF
                    # broadcast per-token (offset) indices across partitions:
                    # b3[p, t] = idx of the bin-block partition p belongs to
                    b3 = psb3.tile([128, HALF], f32, tag="b3")
                    nc.tensor.matmul(
                        b3[:, :], sel3[:, :], idx3[:, t0 : t0 + HALF],
                        start=True, stop=True,
                    )
                    # ctT[bin, t] = (b3[bin, t] == bin)  -- 0/1, exact in bf16
                    nc.vector.tensor_scalar(
                        out=ctT[:, h * HALF : (h + 1) * HALF],
                        in0=b3[:, :],
                        scalar1=iota[:, :],
                        scalar2=None,
                        op0=Alu.is_equal,
                    )
                    # token-major one-hot for the output: PE transpose
                    ctps = psct.tile([128, HALF], bf16, tag="ctps")
                    for j4 in range(4):
                        j = h * 4 + j4
                        nc.tensor.transpose(
                            ctps[:, j4 * 128 : (j4 + 1) * 128],
                            ctT[:, j * 128 : (j + 1) * 128],
                            ident[:, :],
                        )
                    nc.scalar.copy(
                        out=st3[:, h * 4 : (h + 1) * 4, 128:256],
                        in_=ctps[:, :].rearrange("p (j f) -> p j f", f=128),
                    )
                    # gather of W.T rows: Cct = ctT.T @ (w_hi + w_lo)
                    cc = pscc.tile([128, HALF], f32, tag="cc")
                    for j4 in range(4):
                        j = h * 4 + j4
                        lhsT = ctT[:, j * 128 : (j + 1) * 128]
                        oslice = cc[:, j4 * 128 : (j4 + 1) * 128]
                        nc.tensor.matmul(oslice, lhsT, whi[:, :], start=True, stop=False)
                        nc.tensor.matmul(oslice, lhsT, wlo[:, :], start=False, stop=True)
                    # theta = vt * Cct
                    nc.vector.tensor_tensor(
                        out=st3[:, h * 4 : (h + 1) * 4, 0:128],
                        in0=vt_sb[:, h * HALF : (h + 1) * HALF].rearrange(
                            "p (j f) -> p j f", f=128
                        ),
                        in1=cc[:, :].rearrange("p (j f) -> p j f", f=128),
                        op=Alu.mult,
                    )
                nc.sync.dma_start(out=out_ext[c], in_=stage[:, :])

    nc.compile()
    return nc


def _get_compiled(repeats=1):
    if repeats not in _compiled:
        _compiled[repeats] = _build_program(repeats)
    return _compiled[repeats]


def _host_prep(vt, rgap, sgap, pcount, W):
    import concourse.mybir as mybir

    bf16 = mybir.dt.np(mybir.dt.bfloat16)

    vt = np.ascontiguousarray(np.asarray(vt), dtype=np.float32)
    W = np.asarray(W, dtype=np.float32)
    rgap = np.asarray(rgap).astype(np.int64)
    sgap = np.asarray(sgap).astype(np.int64)
    pcount = np.asarray(pcount).astype(np.int64)

    Wt = np.ascontiguousarray(W.T)  # [bin, emb]
    w_hi = Wt.astype(bf16)
    w_lo = (Wt - w_hi.astype(np.float32)).astype(bf16)

    sel3 = np.zeros((3, 128), dtype=np.float32)
    sel3[0, :NUM_RGAP] = 1.0
    sel3[1, NUM_RGAP : NUM_RGAP + NUM_SGAP] = 1.0
    sel3[2, NUM_RGAP + NUM_SGAP :] = 1.0
    sel3 = sel3.astype(bf16)

    iota_col = np.arange(128, dtype=np.float32).reshape(128, 1)
    ident = np.eye(128, dtype=np.float32).astype(bf16)

    # combined bin indices, int values < 128 (exact in bf16)
    idx = np.stack(
        [rgap, NUM_RGAP + sgap, NUM_RGAP + NUM_SGAP + pcount]
    ).astype(np.float32)  # [3, B, S]

    in_maps = []
    for core in range(NCORES):
        r0 = core * ROWS_PER_CORE
        vt_c = vt[r0 : r0 + ROWS_PER_CORE].reshape(NCHUNK, 128, CHUNK)
        # token order within a chunk column-space: col j*128+p <-> token 8p+j
        idx_c = (
            idx[:, r0 : r0 + ROWS_PER_CORE, :]
            .reshape(3, NCHUNK, 128, JT)
            .transpose(0, 1, 3, 2)
            .reshape(3, T_CORE)
        ).astype(bf16)
        in_maps.append(
            {
                "vt": vt_c,
                "idx3": np.ascontiguousarray(idx_c),
                "w_hi": w_hi,
                "w_lo": w_lo,
                "sel3": sel3,
                "iota_col": iota_col,
                "ident": ident,
            }
        )
    return in_maps


def _run(in_maps, trace=False, repeats=1):
    from concourse.bass_utils import run_bass_kernel_spmd

    nc = _get_compiled(repeats)
    return run_bass_kernel_spmd(nc, in_maps, list(range(NCORES)), trace=trace)


def kernel(vt, rgap, sgap, pcount, W):
    in_maps = _host_prep(vt, rgap, sgap, pcount, W)
    res = _run(in_maps)
    outs = []
    for core in range(NCORES):
        o = res.results[core]["out"]  # [NCHUNK, 128, 2048]
        # [c2, p, j*256+f] -> token c2*1024 + 8p + j
        o = o.reshape(NCHUNK, 128 * JT, 2 * EMB).reshape(ROWS_PER_CORE, S, 2 * EMB)
        outs.append(o)
    return np.ascontiguousarray(np.concatenate(outs, axis=0))


if __name__ == "__main__":
    rng = np.random.default_rng(0)
    vt = rng.standard_normal((B, S, EMB), dtype=np.float32)
    rgap = rng.integers(0, NUM_RGAP, (B, S))
    sgap = rng.integers(0, NUM_SGAP, (B, S))
    pcount = rng.integers(0, NUM_PCOUNT, (B, S))
    W = (rng.standard_normal((EMB, NTOTAL)) * 0.05).astype(np.float32)
    out = kernel(vt, rgap, sgap, pcount, W)
    print(out.shape, out.dtype)


# revision 16
# speedup vs baseline: 1892.6727x; 1892.6727x over previous
"""Trainium2 Bass kernel for the CIntegration embedding-lookup module.

reference semantics (all fp32):
    ct    = concat(one_hot(rgap, 32), one_hot(sgap, 32), one_hot(pcount, 64))  # [B,S,128]
    Cct   = W.T[rgap] + W.T[32+sgap] + W.T[64+pcount]                          # [B,S,128]
    theta = vt * Cct
    out   = concat(theta, ct)                                                  # [B,S,256]

Strategy (8 NeuronCores, data-parallel over the batch dim, W replicated):
  - per core: 32 batch rows = 32768 tokens, processed in 32 chunks of 1024
    tokens. SBUF partition p holds tokens {8p+j, j=0..7} of the chunk so all
    DMAs are fully contiguous per partition.
  - transposed one-hot ctT[bin, tok] built on-chip: a K=3 matmul broadcasts
    the (offset) indices across partitions, then one is_equal against a
    partition-iota yields ctT in bf16 (exact, values 0/1).
  - the 3-row gather of W.T is ctT.T @ W.T, computed as matmuls with the
    one-hot as the stationary operand; W.T is fed as bf16 hi+lo halves
    accumulated in PSUM (exact to ~2^-17 relative).
  - ct (token-major one-hot for the output) is ctT run through the PE
    transpose, then a ScalarE copy (bf16->f32) into the staging tile.
  - theta = vt * Cct is one VectorE multiply per 512 tokens.
  - staging is a [128, 2048] f32 tile per chunk -> a single 1 MiB store DMA.
"""

import sys

import numpy as np

try:  # concourse is on sys.path via sitecustomize in the runtime image;
    import concourse  # noqa: F401  # fall back to known locations otherwise
except ImportError:  # pragma: no cover
    for _p in ("/opt/trn_rl_repo", "/root/.axon_site/_ro/trn_rl_repo"):
        if _p not in sys.path:
            sys.path.insert(0, _p)

B, S, EMB = 256, 1024, 128
NUM_RGAP, NUM_SGAP, NUM_PCOUNT = 32, 32, 64
NTOTAL = NUM_RGAP + NUM_SGAP + NUM_PCOUNT  # 128
NCORES = 8
ROWS_PER_CORE = B // NCORES                # 32
T_CORE = ROWS_PER_CORE * S                 # 32768 tokens per core
CHUNK = 1024                               # tokens per chunk
NCHUNK = T_CORE // CHUNK                   # 32
JT = CHUNK // 128                          # 8 token-tiles per chunk
HALF = 512                                 # tokens per PSUM round
NH = CHUNK // HALF                         # 2 halves per chunk

_compiled = {}


def _build_program(repeats=1):
    import concourse.bacc as bacc
    import concourse.mybir as mybir
    from concourse import tile

    f32 = mybir.dt.float32
    bf16 = mybir.dt.bfloat16
    Alu = mybir.AluOpType

    nc = bacc.Bacc(None)

    vt_in = nc.declare_dram_parameter("vt", [NCHUNK, 128, CHUNK], f32, isOutput=False)
    idx3_in = nc.declare_dram_parameter("idx3", [3, T_CORE], bf16, isOutput=False)
    # w_hi | w_lo | identity packed side by side -> one preamble DMA
    wpack_in = nc.declare_dram_parameter("wpack", [128, 384], bf16, isOutput=False)
    sel_in = nc.declare_dram_parameter("sel3", [3, 128], bf16, isOutput=False)
    iota_in = nc.declare_dram_parameter("iota_col", [128, 1], f32, isOutput=False)
    out_ext = nc.declare_dram_parameter(
        "out", [NCHUNK, 128, 2 * CHUNK], f32, isOutput=True
    )

    with tile.TileContext(nc) as tc:
        with (
            tc.tile_pool(name="consts", bufs=1) as consts,
            tc.tile_pool(name="vt", bufs=6) as vtp,
            tc.tile_pool(name="stage", bufs=6) as stp,
            tc.tile_pool(name="ctT", bufs=2) as ctp,
            tc.tile_pool(name="ps_b3", bufs=2, space="PSUM") as psb3,
            tc.tile_pool(name="ps_ct", bufs=2, space="PSUM") as psct,
            tc.tile_pool(name="ps_cc", bufs=2, space="PSUM") as pscc,
        ):
            wpack = consts.tile([128, 384], bf16, tag="wpack")
            whi = wpack[:, 0:128]
            wlo = wpack[:, 128:256]
            ident = wpack[:, 256:384]
            sel3 = consts.tile([3, 128], bf16, tag="sel3")
            iota = consts.tile([128, 1], f32, tag="iota")
            idx3 = consts.tile([3, T_CORE], bf16, tag="idx3")
            # constants go via SWDGE so the SP HWDGE ring starts streaming vt
            # immediately (they are only needed once compute begins)
            nc.gpsimd.dma_start(out=wpack[:, :], in_=wpack_in[:, :])
            nc.gpsimd.dma_start(out=sel3[:, :], in_=sel_in[:, :])
            nc.gpsimd.dma_start(out=iota[:, :], in_=iota_in[:, :])
            # idx3 gates all compute: put it on the ACT ring, which is idle
            # until the first store
            nc.scalar.dma_start(out=idx3[:, :], in_=idx3_in[:, :])

            for c in [ci for _ in range(repeats) for ci in range(NCHUNK)]:
                vt_sb = vtp.tile([128, CHUNK], f32, tag="vt")
                nc.sync.dma_start(out=vt_sb[:, :], in_=vt_in[c])
                stage = stp.tile([128, 2 * CHUNK], f32, tag="stage")
                st3 = stage[:, :].rearrange("p (j f) -> p j f", f=256)
                ctT = ctp.tile([128, CHUNK], bf16, tag="ctT")

                for h in range(NH):
                    t0 = c * CHUNK + h * HALF
                    # broadcast per-token (offset) indices across partitions:
                    # b3[p, t] = idx of the bin-block partition p belongs to
                    b3 = psb3.tile([128, HALF], f32, tag="b3")
                    nc.tensor.matmul(
                        b3[:, :], sel3[:, :], idx3[:, t0 : t0 + HALF],
                        start=True, stop=True,
                    )
                    # ctT[bin, t] = (b3[bin, t] == bin)  -- 0/1, exact in bf16
                    nc.vector.tensor_scalar(
                        out=ctT[:, h * HALF : (h + 1) * HALF],
                        in0=b3[:, :],
                        scalar1=iota[:, :],
                        scalar2=None,
                        op0=Alu.is_equal,
                    )
                    # token-major one-hot for the output: PE transpose
                    ctps = psct.tile([128, HALF], bf16, tag="ctps")
                    for j4 in range(4):
                        j = h * 4 + j4
                        nc.tensor.transpose(
                            ctps[:, j4 * 128 : (j4 + 1) * 128],
                            ctT[:, j * 128 : (j + 1) * 128],
                            ident[:, :],
                        )
                    nc.scalar.copy(
                        out=st3[:, h * 4 : (h + 1) * 4, 128:256],
                        in_=ctps[:, :].rearrange("p (j f) -> p j f", f=128),
                    )
                    # gather of W.T rows: Cct = ctT.T @ (w_hi + w_lo)
                    cc = pscc.tile([128, HALF], f32, tag="cc")
                    for j4 in range(4):
                        j = h * 4 + j4
                        lhsT = ctT[:, j * 128 : (j + 1) * 128]
                        oslice = cc[:, j4 * 128 : (j4 + 1) * 128]
                        nc.tensor.matmul(oslice, lhsT, whi[:, :], start=True, stop=False)
                        nc.tensor.matmul(oslice, lhsT, wlo[:, :], start=False, stop=True)
                    # theta = vt * Cct
                    nc.vector.tensor_tensor(
                        out=st3[:, h * 4 : (h + 1) * 4, 0:128],
                        in0=vt_sb[:, h * HALF : (h + 1) * HALF].rearrange(
                            "p (j f) -> p j f", f=128
                        ),
                        in1=cc[:, :].rearrange("p (j f) -> p j f", f=128),
                        op=Alu.mult,
                    )
                # output store on the ACT HWDGE ring so it overlaps with the
                # vt loads issued on the SP ring
                nc.scalar.dma_start(out=out_ext[c], in_=stage[:, :])

    nc.compile()
    return nc


def _get_compiled(repeats=1):
    if repeats not in _compiled:
        _compiled[repeats] = _build_program(repeats)
    return _compiled[repeats]


def _host_prep(vt, rgap, sgap, pcount, W):
    import concourse.mybir as mybir

    bf16 = mybir.dt.np(mybir.dt.bfloat16)

    vt = np.ascontiguousarray(np.asarray(vt), dtype=np.float32)
    W = np.asarray(W, dtype=np.float32)
    rgap = np.asarray(rgap).astype(np.int64)
    sgap = np.asarray(sgap).astype(np.int64)
    pcount = np.asarray(pcount).astype(np.int64)

    Wt = np.ascontiguousarray(W.T)  # [bin, emb]
    w_hi = Wt.astype(bf16)
    w_lo = (Wt - w_hi.astype(np.float32)).astype(bf16)
    ident = np.eye(128, dtype=np.float32).astype(bf16)
    wpack = np.ascontiguousarray(
        np.concatenate([w_hi, w_lo, ident], axis=1)
    )  # [128, 384]

    sel3 = np.zeros((3, 128), dtype=np.float32)
    sel3[0, :NUM_RGAP] = 1.0
    sel3[1, NUM_RGAP : NUM_RGAP + NUM_SGAP] = 1.0
    sel3[2, NUM_RGAP + NUM_SGAP :] = 1.0
    sel3 = sel3.astype(bf16)

    iota_col = np.arange(128, dtype=np.float32).reshape(128, 1)

    # combined bin indices, int values < 128 (exact in bf16)
    idx = np.stack(
        [rgap, NUM_RGAP + sgap, NUM_RGAP + NUM_SGAP + pcount]
    ).astype(np.float32)  # [3, B, S]

    in_maps = []
    for core in range(NCORES):
        r0 = core * ROWS_PER_CORE
        vt_c = vt[r0 : r0 + ROWS_PER_CORE].reshape(NCHUNK, 128, CHUNK)
        # token order within a chunk column-space: col j*128+p <-> token 8p+j
        idx_c = (
            idx[:, r0 : r0 + ROWS_PER_CORE, :]
            .reshape(3, NCHUNK, 128, JT)
            .transpose(0, 1, 3, 2)
            .reshape(3, T_CORE)
        ).astype(bf16)
        in_maps.append(
            {
                "vt": vt_c,
                "idx3": np.ascontiguousarray(idx_c),
                "wpack": wpack,
                "sel3": sel3,
                "iota_col": iota_col,
            }
        )
    return in_maps


def _run(in_maps, trace=False, repeats=1):
    from concourse.bass_utils import run_bass_kernel_spmd

    nc = _get_compiled(repeats)
    return run_bass_kernel_spmd(nc, in_maps, list(range(NCORES)), trace=trace)


def kernel(vt, rgap, sgap, pcount, W):
    in_maps = _host_prep(vt, rgap, sgap, pcount, W)
    res = _run(in_maps)
    outs = []
    for core in range(NCORES):
        o = res.results[core]["out"]  # [NCHUNK, 128, 2048]
        # [c2, p, j*256+f] -> token c2*1024 + 8p + j
        o = o.reshape(NCHUNK, 128 * JT, 2 * EMB).reshape(ROWS_PER_CORE, S, 2 * EMB)
        outs.append(o)
    return np.ascontiguousarray(np.concatenate(outs, axis=0))


if __name__ == "__main__":
    rng = np.random.default_rng(0)
    vt = rng.standard_normal((B, S, EMB), dtype=np.float32)
    rgap = rng.integers(0, NUM_RGAP, (B, S))
    sgap = rng.integers(0, NUM_SGAP, (B, S))
    pcount = rng.integers(0, NUM_PCOUNT, (B, S))
    W = (rng.standard_normal((EMB, NTOTAL)) * 0.05).astype(np.float32)
    out = kernel(vt, rgap, sgap, pcount, W)
    print(out.shape, out.dtype)
